# revision 1
# baseline (speedup 1.0000x reference)
"""Trainium2 Bass kernel for nn_Attention_77927886618996 — v4.

Math (reference):
  y_t[n,h,l,r] = sum_f x[n,f,r] * T[h,l,f]        for T in {Q, K, D}
  t_n = y_t / ||y_t[n, :, :, :]||                  (norm over ALL heads, l, r)
  S[h,n,m] = sum_{l,r} q_n[n,h,l,r] * k_n[m,h,l,r]
  w = softmax_m(S);  v[n,h,l,r] = sum_m w[h,n,m] * d_n[m,h,l,r]
  out = v.reshape(n, h*l, r)

Sharding: one head per core, x replicated (bf16). Per-n norms couple all
heads -> AllReduces of the per-core sums of squares.

Precision strategy (measured: fp8 noise on the d/es path does NOT wash
out — v is itself a sqrt(N)-suppressed average — while q/k/score noise
enters only through softmax weights and does):
  - projections bf16 (x bf16), scores fp8-DoubleRow, es/dn fp16 V path.
Stage A interleaves qk and the direct-[m,j] d projection per r-pair
(psd passes of [128,128]) so x tiles stream once through a small ring.
Three collectives: h0 (q,k,d), h1 (q,k) early, h1 (d) late — so stage B
blocks unblock as early as possible. Z row sums on DVE (fp16 adds).
"""

import numpy as np
import ml_dtypes

N, F, R, H, L = 2048, 512, 8, 8, 64
NCORES = 8

BF16 = ml_dtypes.bfloat16
F8 = ml_dtypes.float8_e4m3fn

_CACHE = {}


def _build_nc():
    import concourse.bass as bass
    from concourse import bacc, mybir
    import concourse.tile as tile
    from contextlib import ExitStack

    bf = mybir.dt.bfloat16
    f16 = mybir.dt.float16
    f32 = mybir.dt.float32
    f32r = mybir.dt.float32r
    f8 = mybir.dt.float8e4
    DR = mybir.MatmulPerfMode.DoubleRow
    ACT = mybir.ActivationFunctionType

    nc = bacc.Bacc("TRN2", target_bir_lowering=False, debug=False,
                   num_devices=NCORES)

    # xbf[half, r, fp, ft, nc1024] = x[n, f, r], f = ft*128 + fp
    xbf = nc.dram_tensor("xbf", [2, R, 128, 4, 1024], bf,
                         kind="ExternalInput")
    wqkb = nc.dram_tensor("wqkb", [4, 128, 128], bf, kind="ExternalInput")
    wdb = nc.dram_tensor("wdb", [4, 128, 64], bf, kind="ExternalInput")
    vout = nc.dram_tensor("vout", [512, N], bf, kind="ExternalOutput")

    ind_np = np.zeros((128, 2, 32), F8)
    ind_np[0:64, :, 0] = 1
    ind_np[64:128, :, 1] = 1
    ind_dram = nc.inline_tensor(ind_np, "ind2")
    ones16_dram = nc.inline_tensor(np.ones((128, 1), BF16), "ones16")
    ones1_dram = nc.inline_tensor(np.ones((1, 128), np.float32), "ones1")
    ones164_dram = nc.inline_tensor(
        np.full((1, 128), 1.0 / 64.0, np.float32), "ones164")
    warm_dram = nc.inline_tensor(np.zeros((1, 8), np.float32), "warm")

    with tile.TileContext(nc) as tc, ExitStack() as ctx:
        cpool = ctx.enter_context(tc.tile_pool(name="consts", bufs=1))
        xpool = ctx.enter_context(tc.tile_pool(name="xs", bufs=1))
        ypool = ctx.enter_context(tc.tile_pool(name="ys", bufs=1))
        espool = ctx.enter_context(tc.tile_pool(name="es", bufs=1))
        sqpool = ctx.enter_context(tc.tile_pool(name="sqs", bufs=1))
        smallpool = ctx.enter_context(tc.tile_pool(name="small", bufs=1))
        vpool = ctx.enter_context(tc.tile_pool(name="vstage", bufs=1))
        pspool = ctx.enter_context(
            tc.tile_pool(name="ps", bufs=1, space="PSUM"))
        drampool = ctx.enter_context(
            tc.tile_pool(name="dram", bufs=1, space="DRAM"))

        # ---- constants
        wqk_sb = cpool.tile([128, 4, 128], bf, tag="wqk")
        nc.sync.dma_start(wqk_sb[:], wqkb[:].rearrange("t p m -> p t m"))
        wd_sb = cpool.tile([128, 4, 64], bf, tag="wd")
        nc.sync.dma_start(wd_sb[:], wdb[:].rearrange("t p m -> p t m"))
        ind_sb = cpool.tile([128, 2, 32], f8, tag="ind")
        nc.sync.dma_start(ind_sb[:], ind_dram.ap())
        ones16_sb = cpool.tile([128, 1], bf, tag="ones16")
        nc.sync.dma_start(ones16_sb[:], ones16_dram.ap())
        ones1_sb = cpool.tile([1, 128], f32r, tag="ones1")
        nc.sync.dma_start(ones1_sb[:], ones1_dram.ap().bitcast(f32r))
        ones164_sb = cpool.tile([1, 128], f32r, tag="ones164")
        nc.sync.dma_start(ones164_sb[:], ones164_dram.ap().bitcast(f32r))

        # ---- warmup collective: absorbs first-CC cost during x DMA
        warm_out = drampool.tile([1, 8], f32, tag="warmo")
        nc.gpsimd.collective_compute(
            "AllReduce", mybir.AluOpType.add,
            replica_groups=[list(range(NCORES))],
            ins=[warm_dram.ap()], outs=[warm_out.opt()])

        # ---- x ring: bf16 tiles (8KB each); DMAs issued per rp block
        x_sb = [[None] * R for _ in range(2)]

        def x_fetch(h, r, chunked=False):
            t = xpool.tile([128, 4, 1024], bf, tag="x", bufs=8,
                           name=f"x{h}_{r}")
            if chunked:
                for ft in range(4):
                    nc.sync.dma_start(t[:, ft, :], xbf[h, r, :, ft, :])
            else:
                nc.sync.dma_start(t[:], xbf[h, r])
            x_sb[h][r] = t

        for r in range(4):
            x_fetch(0, r, chunked=(r < 2))

        # ---- persistent activations (split per half where halves differ)
        yq8 = [[ypool.tile([128, 2, 1024], f8, tag=f"yq{t}_{h}",
                           name=f"yq{t}_{h}") for h in range(2)]
               for t in range(2)]
        yk8 = [[ypool.tile([128, 2, 1024], f8, tag=f"yk{t}_{h}",
                           name=f"yk{t}_{h}") for h in range(2)]
               for t in range(2)]
        dn16 = [ypool.tile([128, 512], bf, tag=f"dn{m}", name=f"dn{m}")
                for m in range(16)]
        es16 = [[espool.tile([128, 1024], bf, tag=f"es{m}_{nh}",
                             name=f"es{m}_{nh}") for nh in range(2)]
                for m in range(16)]

        # ---- per-half stats
        ssd_h = [smallpool.tile([128, 8], f32, tag=f"ssd{h}",
                                name=f"ssd{h}") for h in range(2)]
        rk_h = [smallpool.tile([128, 8], f32, tag=f"rk{h}",
                               name=f"rk{h}") for h in range(2)]
        rd_h = [smallpool.tile([128, 8], f32, tag=f"rd{h}",
                               name=f"rd{h}") for h in range(2)]
        rnqb_h = [smallpool.tile([128, 1024], bf, tag=f"rnqb{h}",
                                 name=f"rnqb{h}") for h in range(2)]
        rzb_sb = smallpool.tile([128, N], bf, tag="rzb")
        zacc = [smallpool.tile([128, 1024], bf, tag=f"zacc{nh}",
                               name=f"zacc{nh}") for nh in range(2)]
        rz_row = smallpool.tile([1, N], f32r, tag="rz_row")

        # collectives: qk rows per half; d columns per half
        cc0_in = drampool.tile([2, 1024], f32, tag="cc0i")
        cc0_out = drampool.tile([2, 1024], f32, tag="cc0o")
        cq1_in = drampool.tile([2, 1024], f32, tag="cq1i")
        cq1_out = drampool.tile([2, 1024], f32, tag="cq1o")
        cd0_in = drampool.tile([128, 8], f32, tag="cd0i")
        cd0_out = drampool.tile([128, 8], f32, tag="cd0o")
        cd1_in = drampool.tile([128, 8], f32, tag="cd1i")
        cd1_out = drampool.tile([128, 8], f32, tag="cd1o")

        # =========== stage A ===========
        # qk sweep first so the q/k collective triggers early and its
        # latency hides under the d sweep; d collective after d sweep.
        def half_A(h):
            ssa = pspool.tile([32, 1024], f32, tag="ssa", bufs=1,
                              name=f"ssa{h}")
            sq2 = None
            for r in range(R):
                if h == 0 and r < 4:
                    x_fetch(0, r + 4)
                xt = x_sb[h][r]
                rp, rr = r // 2, r % 2
                psq = pspool.tile([128, 1024], f32, tag="big", bufs=2,
                                  name=f"psq{h}_{r}")
                for cs in range(2):
                    csl = slice(cs * 512, (cs + 1) * 512)
                    for ft in range(4):
                        nc.tensor.matmul(psq[:, csl], wqk_sb[:, ft],
                                         xt[:, ft, csl],
                                         start=(ft == 0), stop=(ft == 3),
                                         skip_group_check=True)
                t2, s, ph = r // 4, (r // 2) % 2, r % 2
                psl = slice(ph * 64, (ph + 1) * 64)
                with nc.allow_low_precision(reason="fp8 scores"):
                    nc.vector.tensor_scalar_mul(
                        yq8[t2][h][psl, s, :], psq[0:64, :], 1.0)
                    nc.scalar.activation(
                        yk8[t2][h][psl, s, :], psq[64:128, :],
                        ACT.Copy, bias=0.0, scale=1.0)
                if rr == 0:
                    sq2 = sqpool.tile([128, 2, 1024], f8, tag="sq2",
                                      bufs=2, name=f"sq2_{h}_{rp}")
                with nc.allow_low_precision(reason="fp8 squares"):
                    nc.scalar.activation(sq2[:, rr, :], psq[:],
                                         ACT.Square, bias=0.0,
                                         scale=1.0 / 32.0)
                if rr == 1:
                    for c in range(2):
                        csl = slice(c * 512, (c + 1) * 512)
                        nc.tensor.matmul(ssa[:, csl], ind_sb[:],
                                         sq2[:, :, csl],
                                         start=(rp == 0), stop=(rp == 3),
                                         perf_mode=DR,
                                         skip_group_check=True)
            # ship q/k sums of squares as soon as the qk sweep is done
            ss_sb = smallpool.tile([2, 1024], f32, tag="ss_sb", bufs=1,
                                   name=f"ss_sb{h}")
            nc.vector.tensor_copy(ss_sb[:], ssa[0:2, :])
            qk_in, qk_out = (cc0_in, cc0_out) if h == 0 else (cq1_in,
                                                              cq1_out)
            nc.sync.dma_start(qk_in[0:2, :], ss_sb[:])
            nc.gpsimd.collective_compute(
                "AllReduce", mybir.AluOpType.add,
                replica_groups=[list(range(NCORES))],
                ins=[qk_in.opt()], outs=[qk_out.opt()])
            # d sweep (x tiles stay resident; h1 tiles prefetched here)
            for rp in range(4):
                if h == 0:
                    x_fetch(1, 2 * rp)
                    x_fetch(1, 2 * rp + 1)
                psd = pspool.tile([128, 1024], f32, tag="med", bufs=1,
                                  name=f"psd{h}_{rp}")
                for rr in range(2):
                    r = 2 * rp + rr
                    xt = x_sb[h][r]
                    for ml in range(8):
                        msl = slice(ml * 128, (ml + 1) * 128)
                        jsl = slice(ml * 128 + rr * 64,
                                    ml * 128 + (rr + 1) * 64)
                        for ft in range(4):
                            nc.tensor.matmul(psd[:, jsl],
                                             xt[:, ft, msl],
                                             wd_sb[:, ft],
                                             start=(ft == 0),
                                             stop=(ft == 3),
                                             skip_group_check=True)
                for ml in range(8):
                    mt = h * 8 + ml
                    dj = slice(rp * 128, (rp + 1) * 128)
                    pj = slice(ml * 128, (ml + 1) * 128)
                    with nc.allow_low_precision(reason="f16 dn"):
                        nc.vector.tensor_scalar_mul(
                            dn16[mt][:, dj], psd[:, pj], 1.0)
            # d sums of squares from dn16, then the d collective
            for ml in range(8):
                mt = h * 8 + ml
                dscr = sqpool.tile([128, 512], bf, tag="dscr", bufs=2,
                                   name=f"dscr{mt}")
                with nc.allow_low_precision(reason="bf16 dsq"):
                    nc.vector.tensor_mul(dscr[:], dn16[mt][:],
                                         dn16[mt][:])
                nc.vector.tensor_reduce(ssd_h[h][:, ml:ml + 1], dscr[:],
                                        mybir.AxisListType.X,
                                        mybir.AluOpType.add)
            d_in, d_out = (cd0_in, cd0_out) if h == 0 else (cd1_in,
                                                            cd1_out)
            nc.sync.dma_start(d_in[:], ssd_h[h][:])
            nc.gpsimd.collective_compute(
                "AllReduce", mybir.AluOpType.add,
                replica_groups=[list(range(NCORES))],
                ins=[d_in.opt()], outs=[d_out.opt()])

        # ---- post-collective norms: q/k part (gates stage B)
        def norms_qk(h):
            qk_out = cc0_out if h == 0 else cq1_out
            # rk = 1/(256*sqrt(ss_k)): k row -> columns via PE transposes
            rk_row = smallpool.tile([1, 1024], f32, tag="rk_row", bufs=1,
                                    name=f"rk_row{h}")
            nc.sync.dma_start(rk_row[:], qk_out[1:2, :])
            tps = pspool.tile([128, 8], f32, tag="ssa", bufs=1,
                              name=f"tps{h}")
            for t in range(8):
                nc.tensor.transpose(tps[:, t:t + 1],
                                    rk_row[:, t * 128:(t + 1) * 128],
                                    ones1_sb[:, 0:1].bitcast(f32))
            nc.vector.tensor_copy(rk_h[h][:], tps[:])
            nc.scalar.activation(rk_h[h][:], rk_h[h][:], ACT.Sqrt,
                                 bias=0.0, scale=65536.0)
            nc.vector.reciprocal(rk_h[h][:], rk_h[h][:])
            # rnq row: 0.25/sqrt(ss_q) -> broadcast bf16
            rq_row = smallpool.tile([1, 1024], f32r, tag="rq_row", bufs=1,
                                    name=f"rq_row{h}")
            nc.sync.dma_start(rq_row[:], qk_out[0:1, :].bitcast(f32r))
            with nc.allow_low_precision(reason="f32r row math"):
                nc.scalar.activation(rq_row[:], rq_row[:], ACT.Sqrt,
                                     bias=0.0, scale=16.0)
                nc.vector.reciprocal(rq_row[:], rq_row[:])
            for c in range(2):
                csl = slice(c * 512, (c + 1) * 512)
                bps = pspool.tile([128, 1024], f32, tag="ssa", bufs=1,
                                  name=f"bps{h}_{c}")
                nc.tensor.matmul(bps[:, 0:512], ones1_sb[:],
                                 rq_row[:, csl], start=True, stop=True,
                                 skip_group_check=True)
                with nc.allow_low_precision(reason="rnq bf16"):
                    nc.vector.tensor_copy(rnqb_h[h][:, csl],
                                          bps[:, 0:512])
            with nc.allow_low_precision(reason="fp8 scores"):
                for t2 in range(2):
                    for s in range(2):
                        eng = nc.vector if s == 0 else nc.gpsimd
                        eng.tensor_mul(yq8[t2][h][:, s, :],
                                       yq8[t2][h][:, s, :],
                                       rnqb_h[h][:])

        # d norms: scale dn16 in place (gates stage C only)
        def norms_d(h):
            d_out = cd0_out if h == 0 else cd1_out
            nc.sync.dma_start(rd_h[h][:], d_out[:])
            nc.scalar.activation(rd_h[h][:], rd_h[h][:], ACT.Sqrt,
                                 bias=0.0, scale=1.0 / 4096.0)
            nc.vector.reciprocal(rd_h[h][:], rd_h[h][:])
            for ml in range(8):
                mt = h * 8 + ml
                with nc.allow_low_precision(reason="f16 dn"):
                    nc.vector.tensor_scalar_mul(
                        dn16[mt][:], dn16[mt][:], rd_h[h][:, ml:ml + 1])

        half_A(0)
        half_A(1)
        norms_qk(0)

        # =========== stage B: scores + exp ===========
        def s_block(mt, nh):
            msl = slice((mt % 8) * 128, (mt % 8 + 1) * 128)
            mh = mt // 8
            nsl = slice(nh * 1024, (nh + 1) * 1024)
            sps = pspool.tile([128, 1024], f32,
                              tag=("big" if (mt + nh) % 2 == 0 else "med"),
                              bufs=(2 if (mt + nh) % 2 == 0 else 1),
                              name=f"sps{mt}_{nh}")
            for cs in range(2):
                csl = slice(cs * 512, (cs + 1) * 512)
                for t2 in range(2):
                    nc.tensor.matmul(sps[:, csl], yk8[t2][mh][:, :, msl],
                                     yq8[t2][nh][:, :, csl],
                                     start=(t2 == 0), stop=(t2 == 1),
                                     perf_mode=DR,
                                     skip_group_check=True)
            with nc.allow_low_precision(reason="f16 es"):
                nc.scalar.activation(es16[mt][nh][:], sps[:],
                                     ACT.Exp, bias=0.0,
                                     scale=rk_h[mh][:, mt % 8:mt % 8 + 1])

        for mt in range(8):
            s_block(mt, 0)
        norms_qk(1)
        for mt in range(8, 16):
            s_block(mt, 0)
        # nh1 blocks with Z accumulation interleaved per half
        with nc.allow_low_precision(reason="f16 zsum"):
            for mt in range(16):
                s_block(mt, 1)
                for nh in range(2):
                    if mt == 0:
                        nc.vector.tensor_copy(zacc[nh][:],
                                              es16[0][nh][:])
                    else:
                        nc.vector.tensor_add(zacc[nh][:], zacc[nh][:],
                                             es16[mt][nh][:])
        norms_d(0)
        norms_d(1)

        # =========== stage C: Z scalar sums, then V ===========
        zps_t = []
        for nh in range(2):
            zps = pspool.tile([1, 1024], f32, tag="ssa", bufs=1,
                              name=f"zps{nh}")
            for cs in range(2):
                psl = slice(cs * 512, (cs + 1) * 512)
                nc.tensor.matmul(zps[:, psl], ones16_sb[:],
                                 zacc[nh][:, psl], start=True, stop=True,
                                 skip_group_check=True)
            zps_t.append(zps)

        def v_mm(nh, jt):
            jsl = slice(jt * 128, (jt + 1) * 128)
            vps = pspool.tile([128, 1024], f32, tag="big", bufs=2,
                              name=f"vps{nh}_{jt}")
            for mt in range(16):
                for cs in range(2):
                    psl = slice(cs * 512, (cs + 1) * 512)
                    nc.tensor.matmul(vps[:, psl], dn16[mt][:, jsl],
                                     es16[mt][nh][:, psl],
                                     start=(mt == 0), stop=(mt == 15),
                                     skip_group_check=True)
            return vps

        def v_evac(nh, jt, vps):
            nsl = slice(nh * 1024, (nh + 1) * 1024)
            jsl = slice(jt * 128, (jt + 1) * 128)
            vst = vpool.tile([128, 1024], bf, tag="vst", bufs=2,
                             name=f"vst{nh}_{jt}")
            with nc.allow_low_precision(reason="bf16 out"):
                nc.vector.tensor_mul(vst[:], vps[:], rzb_sb[:, nsl])
            nc.sync.dma_start(vout[jsl, nsl], vst[:])

        def rz_block(nh):
            nsl = slice(nh * 1024, (nh + 1) * 1024)
            with nc.allow_low_precision(reason="f32r recip"):
                nc.vector.reciprocal(rz_row[:, nsl],
                                     zps_t[nh][0:1, :].bitcast(f32r))
            for cs in range(2):
                gsl = slice(nh * 1024 + cs * 512,
                            nh * 1024 + (cs + 1) * 512)
                bzp = pspool.tile([128, 1024], f32, tag="med", bufs=1,
                                  name=f"bzp{nh}_{cs}")
                nc.tensor.matmul(bzp[:, 0:512], ones164_sb[:],
                                 rz_row[:, gsl], start=True, stop=True,
                                 skip_group_check=True)
                with nc.allow_low_precision(reason="bf16 rzb"):
                    nc.vector.tensor_copy(rzb_sb[:, gsl], bzp[:, 0:512])

        # recips (slow DVE) overlap the first V chains; vst gates on rzb
        vps00 = v_mm(0, 0)
        rz_block(0)
        vps01 = v_mm(0, 1)
        rz_block(1)
        v_evac(0, 0, vps00)
        v_evac(0, 1, vps01)
        for jt in range(2, 4):
            v_evac(0, jt, v_mm(0, jt))
        for jt in range(4):
            v_evac(1, jt, v_mm(1, jt))

    nc.compile()
    return nc


def _get_nc():
    if "nc" not in _CACHE:
        _CACHE["nc"] = _build_nc()
    return _CACHE["nc"]


def _prep_inputs(x, Q, K, D):
    """Host-side shard prep. Returns per-core input maps."""
    x = np.asarray(x, dtype=np.float32)
    Q = np.asarray(Q, dtype=np.float32)
    K = np.asarray(K, dtype=np.float32)
    D = np.asarray(D, dtype=np.float32)
    # xbf[half, r, fp, ft, nc] = x[n, f, r], f = ft*128 + fp
    xt = x.transpose(2, 1, 0)                    # (R, F, N)
    xt = xt.reshape(R, 4, 128, 2, 1024)          # (r, ft, fp, half, nc)
    xbf = np.ascontiguousarray(xt.transpose(3, 0, 2, 1, 4)).astype(BF16)

    def wmap(W):  # (64 or 128, F) -> [ft, fp, m]
        m = W.shape[0]
        return np.ascontiguousarray(W.T.reshape(4, 128, m)).astype(BF16)

    in_maps = []
    for c in range(NCORES):
        wqk = np.concatenate([Q[c], K[c]], axis=0)  # (128, F)
        in_maps.append({"xbf": xbf, "wqkb": wmap(wqk), "wdb": wmap(D[c])})
    return in_maps


def _assemble(results):
    """Per-core (512, 2048) V^T (j = r*64+l) -> full (N, H*L, R) output."""
    out = np.empty((N, H * L, R), dtype=np.float32)
    for c in range(NCORES):
        vT = np.asarray(results[c]["vout"], dtype=np.float32)
        out[:, c * L:(c + 1) * L, :] = vT.reshape(R, L, N).transpose(2, 1, 0)
    return out


def kernel(x, Q, K, D, _trace=False):
    from concourse.bass_utils import run_bass_kernel_spmd

    nc = _get_nc()
    in_maps = _prep_inputs(x, Q, K, D)
    res = run_bass_kernel_spmd(nc, in_maps, core_ids=list(range(NCORES)),
                               trace=_trace)
    out = _assemble(res.results)
    if _trace:
        _CACHE["last_results"] = res
    return out

